# revision 7
# baseline (speedup 1.0000x reference)
"""Trainium2 Bass kernel for nn_Attention_2826088481156.

Dense transformer attention block:
    qkv = x @ W_qkv.T + b_qkv            [B,T,3,H,D]
    scores = q k^T * SCALE + log(clip(cutoffs, 1e-15))
    attn = softmax(scores)
    out  = (attn @ v) @ W_out.T + b_out

Sharding (8 NeuronCores): data-parallel over B (=2), tensor-parallel over
heads (16 heads -> 4 per core).  Each core computes the full attention for
its 4 heads and a partial output projection over its 256 channels; the
host sums the 4 partials per batch and adds the (host-folded) biases.

Key algebra used on device:
    softmax(s + log c) = (c * exp(s)) / sum_k(c * exp(s))   [no log, no max]
    attn @ [V | 1] gives both the weighted values and the softmax
    denominator (row r of the PSUM accumulator), so normalization is a
    reciprocal + partition-broadcast + multiply at the end.
    b_v and b_out never enter the nonlinearity; they are folded on host:
    y += W_out @ b_v + b_out.

Everything on device is computed in a transposed [channel, token] layout so
that every matmul has its contraction dim on partitions; the host feeds
pre-transposed fp16 inputs (layout prep is part of sharding).
"""

import numpy as np

import concourse.bass as bass
import concourse.tile as tile
from concourse import bacc, mybir
from concourse.bass_utils import run_bass_kernel_spmd
from concourse.bass_interp import get_hw_module

F16 = mybir.dt.float16
F32 = mybir.dt.float32
Exp = mybir.ActivationFunctionType.Exp

DIM = 1024
H = 16
D = 64
B = 2
T = 2048
SCALE = 0.125          # 1/sqrt(64)
HPC = 4                # heads per core
CH = HPC * D           # 256 channels per core
NCORES = 8

_cache = {}


def build_kernel(t=T, compile_hw=True):
    """Build (and bacc-compile) the single-core SPMD program."""
    nc = bacc.Bacc("TRN2", target_bir_lowering=False, debug=False,
                   num_devices=NCORES)

    n_cb = DIM // 128          # 8 contraction blocks for projections
    n_kb = t // 128            # key blocks
    QW = 1024 if t >= 1024 else t   # query chunk width
    n_qcc = t // QW            # query chunks

    xT = nc.dram_tensor("xT", [DIM, t], F16, kind="ExternalInput")
    cT = nc.dram_tensor("cT", [t, t], F16, kind="ExternalInput")
    wqkT = nc.dram_tensor("wqkT", [DIM, 2 * CH], F16, kind="ExternalInput")
    wvT = nc.dram_tensor("wvT", [DIM, CH], F16, kind="ExternalInput")
    woT = nc.dram_tensor("woT", [CH, DIM], F16, kind="ExternalInput")
    bqk = nc.dram_tensor("bqk", [128, 4], F32, kind="ExternalInput")
    yT = nc.dram_tensor("yT", [DIM, t], F32, kind="ExternalOutput")

    with tile.TileContext(nc) as tc:
        from contextlib import ExitStack
        with ExitStack() as ctx:
            const = ctx.enter_context(tc.tile_pool(name="const", bufs=1))
            qkp = ctx.enter_context(tc.tile_pool(name="qkT", bufs=1))
            vp = ctx.enter_context(tc.tile_pool(name="v65", bufs=1))
            otp = ctx.enter_context(tc.tile_pool(name="ot", bufs=1))

            wqk_sb = []
            wv_sb = []
            for cb in range(n_cb):
                w1 = const.tile([128, 2 * CH], F16, tag=f"wqk{cb}", name=f"wqk{cb}")
                nc.sync.dma_start(w1[:], wqkT[cb * 128:(cb + 1) * 128, :])
                wqk_sb.append(w1)
                w2 = const.tile([128, CH], F16, tag=f"wv{cb}", name=f"wv{cb}")
                nc.sync.dma_start(w2[:], wvT[cb * 128:(cb + 1) * 128, :])
                wv_sb.append(w2)
            wo_sb = []
            for j in range(2):
                w3 = const.tile([128, DIM], F16, tag=f"wo{j}", name=f"wo{j}")
                nc.sync.dma_start(w3[:], woT[j * 128:(j + 1) * 128, :])
                wo_sb.append(w3)
            bqk_sb = const.tile([128, 4], F32, tag="bqk")
            nc.sync.dma_start(bqk_sb[:], bqk[:, :])

            # qkT_sb[j]: j=0 Q heads 0-1, j=1 Q heads 2-3, j=2 K heads 0-1, j=3 K heads 2-3
            qkT_sb = [qkp.tile([128, t], F16, tag=f"qk{j}", name=f"qkT{j}") for j in range(4)]
            # v65_sb[tb][:, h, 0:64] = V head h rows tb; [:, h, 64] = 1.0
            v65_sb = [vp.tile([128, HPC, 65], F16, tag=f"v{tb}", name=f"v65_{tb}") for tb in range(n_kb)]
            # OT_sb[j]: normalized attention output^T, heads (2j, 2j+1)
            ot_sb = [otp.tile([128, t], F16, tag=f"ot{j}", name=f"ot{j}") for j in range(2)]

            with tc.tile_pool(name="xTp", bufs=1) as xp:
                xT_sb = []
                for cb in range(n_cb):
                    xt = xp.tile([128, t], F16, tag=f"x{cb}", name=f"xt{cb}")
                    nc.sync.dma_start(xt[:], xT[cb * 128:(cb + 1) * 128, :])
                    xT_sb.append(xt)

                # ---- Stage A: qk^T = W_qk @ x^T (+bias) ----
                with tc.tile_pool(name="psA", bufs=2, space="PSUM") as psA:
                    for ob in range(4):
                        for tbb in range(t // QW):
                            p = psA.tile([128, QW], F32, tag="pa", name="pa")
                            for cb in range(n_cb):
                                for ns in range(QW // 512):
                                    nc.tensor.matmul(
                                        p[:, ns * 512:(ns + 1) * 512],
                                        wqk_sb[cb][:, ob * 128:(ob + 1) * 128],
                                        xT_sb[cb][:, tbb * QW + ns * 512: tbb * QW + (ns + 1) * 512],
                                        start=(cb == 0), stop=(cb == n_cb - 1))
                            nc.vector.tensor_scalar_add(
                                qkT_sb[ob][:, tbb * QW:(tbb + 1) * QW],
                                p[:], bqk_sb[:, ob:ob + 1])

                # ---- Stage B: V = x @ W_v^T (natural layout, interleaved 65) ----
                with tc.tile_pool(name="psB", bufs=2, space="PSUM") as psB:
                    for tb in range(n_kb):
                        p = psB.tile([128, CH], F32, tag="pb", name="pb")
                        for cb in range(n_cb):
                            nc.tensor.matmul(
                                p[:], xT_sb[cb][:, tb * 128:(tb + 1) * 128],
                                wv_sb[cb][:], start=(cb == 0), stop=(cb == n_cb - 1))
                        nc.vector.memset(v65_sb[tb][:, :, 64:65], 1.0)
                        nc.vector.tensor_copy(
                            v65_sb[tb][:, :, 0:64],
                            p[:].rearrange("p (h d) -> p h d", d=D))

            # ---- Stage C: attention + output projection ----
            with tc.tile_pool(name="cTp", bufs=min(n_kb + 4, 2 * n_kb)) as cp, \
                 tc.tile_pool(name="ep", bufs=3) as ep, \
                 tc.tile_pool(name="pp", bufs=3) as pp, \
                 tc.tile_pool(name="rp", bufs=2) as rp, \
                 tc.tile_pool(name="rbp", bufs=2) as rbp, \
                 tc.tile_pool(name="tmpp", bufs=2) as tmpp, \
                 tc.tile_pool(name="yp", bufs=4) as yp, \
                 tc.tile_pool(name="psS", bufs=2, space="PSUM") as psS, \
                 tc.tile_pool(name="psO", bufs=2, space="PSUM") as psO:

                for qcc in range(n_qcc):
                    q0 = qcc * QW
                    cT_tiles = []
                    for kb in range(n_kb):
                        ct = cp.tile([128, QW], F16, tag="ct", name=f"ct{kb}")
                        nc.sync.dma_start(ct[:], cT[kb * 128:(kb + 1) * 128, q0:q0 + QW])
                        cT_tiles.append(ct)

                    for h in range(HPC):
                        j, bp = h // 2, (h % 2) * 64
                        O = psO.tile([128, QW], F32, tag="O", name="O")
                        for kb in range(n_kb):
                            S = psS.tile([128, QW], F32, tag="S", name="S")
                            for ns in range(QW // 512):
                                nc.tensor.matmul(
                                    S[:, ns * 512:(ns + 1) * 512],
                                    qkT_sb[2 + j][bp:bp + 64, kb * 128:(kb + 1) * 128],
                                    qkT_sb[j][bp:bp + 64, q0 + ns * 512:q0 + (ns + 1) * 512],
                                    start=True, stop=True)
                            E = ep.tile([128, QW], F16, tag="E", name="E")
                            nc.scalar.activation(E[:], S[:], Exp, scale=SCALE)
                            P = pp.tile([128, QW], F16, tag="P", name="P")
                            nc.vector.tensor_mul(P[:], E[:], cT_tiles[kb][:])
                            for ns in range(QW // 512):
                                nc.tensor.matmul(
                                    O[0:65, ns * 512:(ns + 1) * 512],
                                    v65_sb[kb][:, h, :],
                                    P[:, ns * 512:(ns + 1) * 512],
                                    start=(kb == 0), stop=(kb == n_kb - 1))
                        # normalization: r is row 64 of O
                        rr = rp.tile([1, QW], F32, tag="rr", name="rr")
                        nc.vector.reciprocal(rr[:], O[64:65, :])
                        rb = rbp.tile([64, QW], F32, tag="rb", name="rb")
                        nc.gpsimd.partition_broadcast(rb[:], rr[:])
                        if bp == 0:
                            nc.vector.tensor_mul(ot_sb[j][0:64, q0:q0 + QW], O[0:64, :], rb[:])
                        else:
                            tmp = tmpp.tile([64, QW], F16, tag="tmp", name="tmp")
                            nc.vector.tensor_mul(tmp[:], O[0:64, :], rb[:])
                            nc.sync.dma_start(ot_sb[j][64:128, q0:q0 + QW], tmp[:])

                    # output projection for this query chunk
                    for ob in range(8):
                        Y = psO.tile([128, QW], F32, tag="O", name="Y")
                        for cb in range(2):
                            for ns in range(QW // 512):
                                nc.tensor.matmul(
                                    Y[:, ns * 512:(ns + 1) * 512],
                                    wo_sb[cb][:, ob * 128:(ob + 1) * 128],
                                    ot_sb[cb][:, q0 + ns * 512:q0 + (ns + 1) * 512],
                                    start=(cb == 0), stop=(cb == 1))
                        ys = yp.tile([128, QW], F32, tag="y", name="ys")
                        nc.vector.tensor_copy(ys[:], Y[:])
                        nc.sync.dma_start(yT[ob * 128:(ob + 1) * 128, q0:q0 + QW], ys[:])

    nc.compile()
    if compile_hw:
        nc.m = get_hw_module(nc.m)
    return nc


def make_in_maps(x, cutoffs, W_qkv, b_qkv, W_out):
    """Host-side sharding: slice + transpose + fp16 cast per core."""
    per_batch = []
    for b in range(B):
        xT_b = np.ascontiguousarray(x[b].T).astype(np.float16)
        cT_b = np.ascontiguousarray(cutoffs[b].T).astype(np.float16)
        per_batch.append((xT_b, cT_b))
    in_maps = []
    for core in range(NCORES):
        b, hg = core // HPC, core % HPC
        ch = slice(hg * CH, (hg + 1) * CH)
        chk = slice(DIM + hg * CH, DIM + (hg + 1) * CH)
        chv = slice(2 * DIM + hg * CH, 2 * DIM + (hg + 1) * CH)
        wqkT = np.ascontiguousarray(
            np.concatenate([W_qkv[ch], W_qkv[chk]], axis=0).T).astype(np.float16)
        wvT = np.ascontiguousarray(W_qkv[chv].T).astype(np.float16)
        woT = np.ascontiguousarray(W_out[:, ch].T).astype(np.float16)
        bqk_pp = np.concatenate([b_qkv[ch], b_qkv[chk]]).reshape(4, 128).T
        in_maps.append({
            "xT": per_batch[b][0], "cT": per_batch[b][1],
            "wqkT": wqkT, "wvT": wvT, "woT": woT,
            "bqk": np.ascontiguousarray(bqk_pp).astype(np.float32),
        })
    return in_maps


def kernel(x, cutoffs, W_qkv, b_qkv, W_out, b_out):
    x = np.asarray(x, dtype=np.float32)
    cutoffs = np.asarray(cutoffs, dtype=np.float32)
    W_qkv = np.asarray(W_qkv, dtype=np.float32)
    b_qkv = np.asarray(b_qkv, dtype=np.float32)
    W_out = np.asarray(W_out, dtype=np.float32)
    b_out = np.asarray(b_out, dtype=np.float32)

    if "nc" not in _cache:
        _cache["nc"] = build_kernel()
    nc = _cache["nc"]

    in_maps = make_in_maps(x, cutoffs, W_qkv, b_qkv, W_out)
    res = None
    last_err = None
    for attempt in range(3):
        try:
            res = run_bass_kernel_spmd(nc, in_maps, core_ids=list(range(NCORES)),
                                       trace=False)
            break
        except Exception as e:  # transient NRT/axon failures: retry
            last_err = e
            import time
            time.sleep(5)
    if res is None:
        raise last_err

    y = np.zeros((B, T, DIM), dtype=np.float32)
    for core in range(NCORES):
        b = core // HPC
        y[b] += res.results[core]["yT"].T
    bias_vec = W_out @ b_qkv[2 * DIM:] + b_out
    y += bias_vec[None, None, :]
    return y


# revision 8
# speedup vs baseline: 206.5508x; 206.5508x over previous
"""Trainium2 Bass kernel for nn_Attention_2826088481156.

Dense transformer attention block:
    qkv = x @ W_qkv.T + b_qkv            [B,T,3,H,D]
    scores = q k^T * SCALE + log(clip(cutoffs, 1e-15))
    attn = softmax(scores)
    out  = (attn @ v) @ W_out.T + b_out

Sharding (8 NeuronCores): data-parallel over B (=2), tensor-parallel over
heads (16 heads -> 4 per core).  Each core computes the full attention for
its 4 heads and a partial output projection over its 256 channels; the
host sums the 4 partials per batch and adds the (host-folded) biases.

Key algebra used on device:
    softmax(s + log c) = (c * exp(s)) / sum_k(c * exp(s))   [no log, no max]
    attn @ [V | 1] gives both the weighted values and the softmax
    denominator (row r of the PSUM accumulator), so normalization is a
    reciprocal + partition-broadcast + multiply at the end.
    b_v and b_out never enter the nonlinearity; they are folded on host:
    y += W_out @ b_v + b_out.

Everything on device is computed in a transposed [channel, token] layout so
that every matmul has its contraction dim on partitions; the host feeds
pre-transposed fp16 inputs (layout prep is part of sharding).
"""

import numpy as np

import concourse.bass as bass
import concourse.tile as tile
from concourse import bacc, mybir
from concourse.bass_utils import run_bass_kernel_spmd
from concourse.bass_interp import get_hw_module

F16 = mybir.dt.float16
F32 = mybir.dt.float32
Exp = mybir.ActivationFunctionType.Exp

DIM = 1024
H = 16
D = 64
B = 2
T = 2048
SCALE = 0.125          # 1/sqrt(64)
HPC = 4                # heads per core
CH = HPC * D           # 256 channels per core
NCORES = 8

_cache = {}


def build_kernel(t=T, compile_hw=True, loop_reps=0):
    """Build (and bacc-compile) the single-core SPMD program.

    loop_reps > 0 wraps the whole body in a hardware loop (for timing:
    the body runs loop_reps times per NEFF execution)."""
    from contextlib import ExitStack, nullcontext
    nc = bacc.Bacc("TRN2", target_bir_lowering=False, debug=False,
                   num_devices=NCORES)

    n_cb = DIM // 128          # 8 contraction blocks for projections
    n_kb = t // 128            # key blocks
    QW = 1024 if t >= 1024 else t   # query chunk width
    n_qcc = t // QW            # query chunks

    xT = nc.dram_tensor("xT", [DIM, t], F16, kind="ExternalInput")
    cT = nc.dram_tensor("cT", [t, t], F16, kind="ExternalInput")
    wqkT = nc.dram_tensor("wqkT", [DIM, 2 * CH], F16, kind="ExternalInput")
    wvT = nc.dram_tensor("wvT", [DIM, CH], F16, kind="ExternalInput")
    woT = nc.dram_tensor("woT", [CH, DIM], F16, kind="ExternalInput")
    bqk = nc.dram_tensor("bqk", [128, 4], F32, kind="ExternalInput")
    yT = nc.dram_tensor("yT", [DIM, t], F32, kind="ExternalOutput")

    with tile.TileContext(nc) as tc:
        loop_ctx = tc.For_i(0, loop_reps, 1) if loop_reps else nullcontext()
        with loop_ctx, ExitStack() as ctx:
            const = ctx.enter_context(tc.tile_pool(name="const", bufs=1))
            qkp = ctx.enter_context(tc.tile_pool(name="qkT", bufs=1))
            vp = ctx.enter_context(tc.tile_pool(name="v65", bufs=1))
            otp = ctx.enter_context(tc.tile_pool(name="ot", bufs=1))

            wqk_sb = []
            wv_sb = []
            for cb in range(n_cb):
                w1 = const.tile([128, 2 * CH], F16, tag=f"wqk{cb}", name=f"wqk{cb}")
                nc.sync.dma_start(w1[:], wqkT[cb * 128:(cb + 1) * 128, :])
                wqk_sb.append(w1)
                w2 = const.tile([128, CH], F16, tag=f"wv{cb}", name=f"wv{cb}")
                nc.sync.dma_start(w2[:], wvT[cb * 128:(cb + 1) * 128, :])
                wv_sb.append(w2)
            wo_sb = []
            for j in range(2):
                w3 = const.tile([128, DIM], F16, tag=f"wo{j}", name=f"wo{j}")
                nc.sync.dma_start(w3[:], woT[j * 128:(j + 1) * 128, :])
                wo_sb.append(w3)
            bqk_sb = const.tile([128, 4], F32, tag="bqk")
            nc.sync.dma_start(bqk_sb[:], bqk[:, :])

            # qkT_sb[j]: j=0 Q heads 0-1, j=1 Q heads 2-3, j=2 K heads 0-1, j=3 K heads 2-3
            qkT_sb = [qkp.tile([128, t], F16, tag=f"qk{j}", name=f"qkT{j}") for j in range(4)]
            # v65_sb[tb][:, h, 0:64] = V head h rows tb; [:, h, 64] = 1.0
            v65_sb = [vp.tile([128, HPC, 65], F16, tag=f"v{tb}", name=f"v65_{tb}") for tb in range(n_kb)]
            # OT_sb[j]: normalized attention output^T, heads (2j, 2j+1)
            ot_sb = [otp.tile([128, t], F16, tag=f"ot{j}", name=f"ot{j}") for j in range(2)]

            with tc.tile_pool(name="xTp", bufs=1) as xp:
                xT_sb = []
                for cb in range(n_cb):
                    xt = xp.tile([128, t], F16, tag=f"x{cb}", name=f"xt{cb}")
                    nc.sync.dma_start(xt[:], xT[cb * 128:(cb + 1) * 128, :])
                    xT_sb.append(xt)

                # ---- Stage A: qk^T = W_qk @ x^T (+bias) ----
                with tc.tile_pool(name="psA", bufs=2, space="PSUM") as psA:
                    for ob in range(4):
                        for tbb in range(t // QW):
                            p = psA.tile([128, QW], F32, tag="pa", name="pa")
                            for cb in range(n_cb):
                                for ns in range(QW // 512):
                                    nc.tensor.matmul(
                                        p[:, ns * 512:(ns + 1) * 512],
                                        wqk_sb[cb][:, ob * 128:(ob + 1) * 128],
                                        xT_sb[cb][:, tbb * QW + ns * 512: tbb * QW + (ns + 1) * 512],
                                        start=(cb == 0), stop=(cb == n_cb - 1))
                            nc.vector.tensor_scalar_add(
                                qkT_sb[ob][:, tbb * QW:(tbb + 1) * QW],
                                p[:], bqk_sb[:, ob:ob + 1])

                # ---- Stage B: V = x @ W_v^T (natural layout, interleaved 65) ----
                with tc.tile_pool(name="psB", bufs=2, space="PSUM") as psB:
                    for tb in range(n_kb):
                        p = psB.tile([128, CH], F32, tag="pb", name="pb")
                        for cb in range(n_cb):
                            nc.tensor.matmul(
                                p[:], xT_sb[cb][:, tb * 128:(tb + 1) * 128],
                                wv_sb[cb][:], start=(cb == 0), stop=(cb == n_cb - 1))
                        nc.vector.memset(v65_sb[tb][:, :, 64:65], 1.0)
                        nc.vector.tensor_copy(
                            v65_sb[tb][:, :, 0:64],
                            p[:].rearrange("p (h d) -> p h d", d=D))

            # ---- Stage C: attention + output projection ----
            with tc.tile_pool(name="cTp", bufs=min(n_kb + 4, 2 * n_kb)) as cp, \
                 tc.tile_pool(name="ep", bufs=3) as ep, \
                 tc.tile_pool(name="pp", bufs=3) as pp, \
                 tc.tile_pool(name="rp", bufs=2) as rp, \
                 tc.tile_pool(name="rbp", bufs=2) as rbp, \
                 tc.tile_pool(name="tmpp", bufs=2) as tmpp, \
                 tc.tile_pool(name="yp", bufs=4) as yp, \
                 tc.tile_pool(name="psS", bufs=2, space="PSUM") as psS, \
                 tc.tile_pool(name="psO", bufs=2, space="PSUM") as psO:

                for qcc in range(n_qcc):
                    q0 = qcc * QW
                    cT_tiles = []
                    for kb in range(n_kb):
                        ct = cp.tile([128, QW], F16, tag="ct", name=f"ct{kb}")
                        nc.sync.dma_start(ct[:], cT[kb * 128:(kb + 1) * 128, q0:q0 + QW])
                        cT_tiles.append(ct)

                    for h in range(HPC):
                        j, bp = h // 2, (h % 2) * 64
                        O = psO.tile([128, QW], F32, tag="O", name="O")
                        for kb in range(n_kb):
                            S = psS.tile([128, QW], F32, tag="S", name="S")
                            for ns in range(QW // 512):
                                nc.tensor.matmul(
                                    S[:, ns * 512:(ns + 1) * 512],
                                    qkT_sb[2 + j][bp:bp + 64, kb * 128:(kb + 1) * 128],
                                    qkT_sb[j][bp:bp + 64, q0 + ns * 512:q0 + (ns + 1) * 512],
                                    start=True, stop=True)
                            E = ep.tile([128, QW], F16, tag="E", name="E")
                            nc.scalar.activation(E[:], S[:], Exp, scale=SCALE)
                            P = pp.tile([128, QW], F16, tag="P", name="P")
                            nc.vector.tensor_mul(P[:], E[:], cT_tiles[kb][:])
                            for ns in range(QW // 512):
                                nc.tensor.matmul(
                                    O[0:65, ns * 512:(ns + 1) * 512],
                                    v65_sb[kb][:, h, :],
                                    P[:, ns * 512:(ns + 1) * 512],
                                    start=(kb == 0), stop=(kb == n_kb - 1))
                        # normalization: r is row 64 of O
                        rr = rp.tile([1, QW], F32, tag="rr", name="rr")
                        nc.vector.reciprocal(rr[:], O[64:65, :])
                        rb = rbp.tile([64, QW], F32, tag="rb", name="rb")
                        nc.gpsimd.partition_broadcast(rb[:], rr[:])
                        if bp == 0:
                            nc.vector.tensor_mul(ot_sb[j][0:64, q0:q0 + QW], O[0:64, :], rb[:])
                        else:
                            tmp = tmpp.tile([64, QW], F16, tag="tmp", name="tmp")
                            nc.vector.tensor_mul(tmp[:], O[0:64, :], rb[:])
                            nc.sync.dma_start(ot_sb[j][64:128, q0:q0 + QW], tmp[:])

                    # output projection for this query chunk
                    for ob in range(8):
                        Y = psO.tile([128, QW], F32, tag="O", name="Y")
                        for cb in range(2):
                            for ns in range(QW // 512):
                                nc.tensor.matmul(
                                    Y[:, ns * 512:(ns + 1) * 512],
                                    wo_sb[cb][:, ob * 128:(ob + 1) * 128],
                                    ot_sb[cb][:, q0 + ns * 512:q0 + (ns + 1) * 512],
                                    start=(cb == 0), stop=(cb == 1))
                        ys = yp.tile([128, QW], F32, tag="y", name="ys")
                        nc.vector.tensor_copy(ys[:], Y[:])
                        nc.sync.dma_start(yT[ob * 128:(ob + 1) * 128, q0:q0 + QW], ys[:])

    nc.compile()
    if compile_hw:
        nc.m = get_hw_module(nc.m)
    return nc


def make_in_maps(x, cutoffs, W_qkv, b_qkv, W_out):
    """Host-side sharding: slice + transpose + fp16 cast per core."""
    per_batch = []
    for b in range(B):
        xT_b = np.ascontiguousarray(x[b].T).astype(np.float16)
        cT_b = np.ascontiguousarray(cutoffs[b].T).astype(np.float16)
        per_batch.append((xT_b, cT_b))
    in_maps = []
    for core in range(NCORES):
        b, hg = core // HPC, core % HPC
        ch = slice(hg * CH, (hg + 1) * CH)
        chk = slice(DIM + hg * CH, DIM + (hg + 1) * CH)
        chv = slice(2 * DIM + hg * CH, 2 * DIM + (hg + 1) * CH)
        wqkT = np.ascontiguousarray(
            np.concatenate([W_qkv[ch], W_qkv[chk]], axis=0).T).astype(np.float16)
        wvT = np.ascontiguousarray(W_qkv[chv].T).astype(np.float16)
        woT = np.ascontiguousarray(W_out[:, ch].T).astype(np.float16)
        bqk_pp = np.concatenate([b_qkv[ch], b_qkv[chk]]).reshape(4, 128).T
        in_maps.append({
            "xT": per_batch[b][0], "cT": per_batch[b][1],
            "wqkT": wqkT, "wvT": wvT, "woT": woT,
            "bqk": np.ascontiguousarray(bqk_pp).astype(np.float32),
        })
    return in_maps


def kernel(x, cutoffs, W_qkv, b_qkv, W_out, b_out):
    x = np.asarray(x, dtype=np.float32)
    cutoffs = np.asarray(cutoffs, dtype=np.float32)
    W_qkv = np.asarray(W_qkv, dtype=np.float32)
    b_qkv = np.asarray(b_qkv, dtype=np.float32)
    W_out = np.asarray(W_out, dtype=np.float32)
    b_out = np.asarray(b_out, dtype=np.float32)

    if "nc" not in _cache:
        _cache["nc"] = build_kernel()
    nc = _cache["nc"]

    in_maps = make_in_maps(x, cutoffs, W_qkv, b_qkv, W_out)
    res = None
    last_err = None
    for attempt in range(3):
        try:
            res = run_bass_kernel_spmd(nc, in_maps, core_ids=list(range(NCORES)),
                                       trace=False)
            break
        except Exception as e:  # transient NRT/axon failures: retry
            last_err = e
            import time
            time.sleep(5)
    if res is None:
        raise last_err

    y = np.zeros((B, T, DIM), dtype=np.float32)
    for core in range(NCORES):
        b = core // HPC
        y[b] += res.results[core]["yT"].T
    bias_vec = W_out @ b_qkv[2 * DIM:] + b_out
    y += bias_vec[None, None, :]
    return y
